# revision 44
# baseline (speedup 1.0000x reference)
"""Sparse (segment + causal) GQA attention on 8 Trainium2 NeuronCores.

Problem: nn_AttentionOp_27719718928719
  query (2, 1024, 32, 128) f32, key/value (2, 1024, 8, 128) f32,
  decoder_segment_ids (2, 1024) i32 (sorted) -> out (2, 1024, 32, 128) f32

Sharding: core c owns kv-head c and its 4 GQA query heads, both batches.
Perfect shard (no replication): Q, K, V, and the output all split 8 ways, and
the compiled program is identical on every core (the block schedule depends
only on the segment ids, which all cores share).

Device algorithm, one unit per (batch b, 128-query block tj) with all 4 heads
fused along the free axis (512 wide everywhere):
  for each valid key block si (causal + segment overlap, host-computed):
    S^T[s, (h,t)] = K[si]^T Q      one fp16 matmul (logit err ~5e-3 rel,
                                   well inside the 2e-2 gate)
    P^T = exp(S^T - 30)            ACT, writes bf16 directly to SBUF
    causal zero (diag blocks only) one Pool affine_select for all 4 heads
    segment-boundary spans         partition-affine selects zero s-rows of
                                   pt for the t-span crossing a boundary
    outT[d, (h,t)] += V[klo:khi]' P^T[klo:khi]   bf16 matmul, PSUM-acc
    sums[1, (h,t)] += 1[klo:khi]' P^T[klo:khi]   bf16 ones matmul
  stage PSUM -> SBUF (DVE), DMA out.
Segment masking costs no PE work at all: each block's t-uniform valid key
range [klo, khi) just narrows the PV/sums contraction, and the few blocks
where a segment boundary crosses the t-range get one cheap gpsimd select
per span instead of mask-bias matmuls.
No softmax max-subtraction: logits are O(+-50) so exp(x-30) stays in range
and exp/sum(exp) matches the reference's stabilized softmax exactly.
Host does the (cheap) normalization out/sums and all layout transposes.
"""

import numpy as np
import ml_dtypes

B, T, S, NQ, NKV, D = 2, 1024, 1024, 32, 8, 128
G = NQ // NKV
BLK = 128
NBLK = S // BLK  # 8
W = G * BLK  # 512: fused 4-head free width
N_CORES = 8
HLOC = NQ // N_CORES  # 4

_compiled_cache = {}

# Test-only knobs (the grading path never sets these): when TRACE is true the
# SPMD run captures an NTFF profile into TRACE_DIR.
TRACE = False
TRACE_DIR = None

# QK product mode: "fp16" = single fp16 matmul (11-bit mantissa, ~5e-3 rel
# out err), "hilo" = 3 bf16 hi/lo matmuls (fp32-grade logits, 3x PE cost).
QK_MODE = "fp16"
# dependency-free PE warmup matmuls (HAM un-throttle) issued before real work
N_WARMUP = 14


def _split_bf16(x):
    hi = x.astype(ml_dtypes.bfloat16)
    lo = (x - hi.astype(np.float32)).astype(ml_dtypes.bfloat16)
    return hi, lo


def _segment_structure(seg):
    """Block schedule for one batch's (sorted) segment ids.

    Returns sched: sched[tj] = list of (si, (klo, khi), sels, diag).
      (klo, khi): the t-uniform envelope of valid key rows in this block;
        PV and the sums matmul contract only over this partition range.
      sels: tuple of (a, e, bound, kind) partition-affine zeroings applied
        to the exp output for t-columns [a, e): kind "ge" keeps s >= bound,
        "lt" keeps s < bound (bound == BLK with "ge" zeroes the whole span).
      diag: in-block causal mask needed (separate affine_select).
    """
    seg = np.asarray(seg)
    t_idx = np.arange(S)
    seg_start = np.zeros(S, np.int64)
    seg_end = np.zeros(S, np.int64)
    for v in np.unique(seg):
        m = seg == v
        lo, hi = np.argmax(m), S - np.argmax(m[::-1])
        seg_start[m], seg_end[m] = lo, hi
    valid_ts = (t_idx[None, :] <= t_idx[:, None]) & (seg[None, :] == seg[:, None])
    v4 = valid_ts.reshape(NBLK, BLK, NBLK, BLK)
    vblk = v4.any(axis=(1, 3))  # [tj, si]

    sched = []
    for tj in range(NBLK):
        entries = []
        sis = [si for si in range(NBLK) if vblk[tj, si]]
        assert sis == list(range(min(sis), max(sis) + 1))
        for si in sis:
            tcols = np.arange(tj * BLK, (tj + 1) * BLK)
            lo_rel = np.clip(seg_start[tcols] - si * BLK, 0, BLK)
            hi_rel = np.clip(seg_end[tcols] - si * BLK, 0, BLK)
            diag = si == tj
            spans = []  # (a, e, lo, hi) with constant valid range [lo, hi)
            a = 0
            for i in range(1, BLK + 1):
                if i == BLK or lo_rel[i] != lo_rel[a] or hi_rel[i] != hi_rel[a]:
                    spans.append((a, i, int(lo_rel[a]), int(hi_rel[a])))
                    a = i
            ne = [s for s in spans if s[2] < s[3]]
            assert ne, (tj, si)
            klo = min(s[2] for s in ne)
            khi = max(s[3] for s in ne)
            sels = []
            for (a2, e2, lo, hi) in spans:
                if lo >= hi:
                    sels.append((a2, e2, BLK, "ge"))  # span entirely invalid
                    continue
                if lo > klo:
                    sels.append((a2, e2, lo, "ge"))
                # on the diagonal the causal mask (s <= t < seg_end)
                # already enforces every span's upper bound
                if hi < khi and not diag:
                    sels.append((a2, e2, hi, "lt"))
            # matmul operand slices must respect PE row-group alignment
            # (base 64 spans groups 2-3; base 32 only group 1): round klo
            # down to a legal base and zero the uncovered rows via a select
            if klo >= 64:
                klo_eff = 64
            elif klo >= 32 and khi <= 64:
                klo_eff = 32
            else:
                klo_eff = 0
            if klo > klo_eff:
                sels.append((0, BLK, klo, "ge"))
            # trailing t-columns whose valid key range is empty need no
            # compute at all: narrow every op of this block to [0, tw)
            tw = BLK
            for (a2, e2, lo, hi) in reversed(spans):
                if lo < hi:
                    break
                tw = a2
            if tw < BLK:
                sels = [s for s in sels if s[0] < tw]
            entries.append((si, (klo_eff, khi), tuple(sels), diag, tw))
        # the first entry opens the PSUM accumulation group and must be
        # full-width so every output column's has_written bit is set; the
        # diagonal block always is (its own position is always valid)
        if entries[0][4] < BLK:
            di = next(i for i, e in enumerate(entries) if e[3])
            entries.insert(0, entries.pop(di))
        sched.append(entries)
    return sched


def _build_program(scheds, qk_mode):
    """Build the SPMD Bass program. scheds indexed by batch."""
    import concourse.bass as bass  # noqa: F401
    from concourse import bacc
    import concourse.mybir as mybir
    import concourse.tile as tile
    from concourse.tile import add_dep_helper

    DT = mybir.dt
    QDT = {"hilo": DT.bfloat16, "fp16": DT.float16}[qk_mode]
    nc = bacc.Bacc(None, target_bir_lowering=False, debug=False)

    qhi_d = nc.dram_tensor("qhi", [B, D, NBLK, HLOC, BLK], QDT, kind="ExternalInput").ap()
    khi_d = nc.dram_tensor("khi", [B, D, S], QDT, kind="ExternalInput").ap()
    if qk_mode == "hilo":
        qlo_d = nc.dram_tensor("qlo", [B, D, NBLK, HLOC, BLK], QDT, kind="ExternalInput").ap()
        klo_d = nc.dram_tensor("klo", [B, D, S], QDT, kind="ExternalInput").ap()
    v_d = nc.dram_tensor("v", [BLK, B, NBLK, D], DT.bfloat16, kind="ExternalInput").ap()
    ones_d = nc.dram_tensor("ones_in", [BLK, 1], DT.bfloat16, kind="ExternalInput").ap()
    outT_d = nc.dram_tensor("outT", [B, NBLK, D, W], DT.float32, kind="ExternalOutput").ap()
    sums_d = nc.dram_tensor("sums", [1, B * NBLK * W], DT.float32, kind="ExternalOutput").ap()

    with tile.TileContext(nc) as tc:
        with (
            tc.tile_pool(name="const", bufs=1) as constp,
            tc.tile_pool(name="qkv", bufs=1) as qkv,
            tc.tile_pool(name="pt", bufs=6) as ptp,
            tc.tile_pool(name="stage", bufs=4) as stage,
            tc.tile_pool(name="sumstage", bufs=3) as sumstage,
            tc.tile_pool(name="ps_s", bufs=3, space="PSUM") as ps_s,
            tc.tile_pool(name="ps_o", bufs=2, space="PSUM") as ps_o,
            tc.tile_pool(name="ps_m", bufs=2, space="PSUM") as ps_m,
            tc.tile_pool(name="ps_w", bufs=1, space="PSUM") as ps_w,
        ):
            # b=0 inputs first so compute can start while b=1 still loads
            k_hi = qkv.tile([D, B, S], QDT)
            v_t = qkv.tile([BLK, B, NBLK, D], DT.bfloat16)
            q_hi = qkv.tile([D, B, NBLK, HLOC, BLK], QDT)
            if qk_mode == "hilo":
                k_lo = qkv.tile([D, B, S], QDT)
                q_lo = qkv.tile([D, B, NBLK, HLOC, BLK], QDT)
            ones_t = constp.tile([BLK, 1], DT.bfloat16)
            exp_bias = constp.tile([BLK, 1], mybir.dt.float32)
            nc.vector.memset(exp_bias, -30.0)

            # PE warmup: the HAM clock gate keeps the PE at 1.2 GHz until it
            # has been busy ~3.4us. Dependency-free dummy matmuls fill the
            # input-DMA wait so the real stream starts at 2.4 GHz.
            warm_w = constp.tile([BLK, BLK], DT.bfloat16)
            warm_ps = ps_w.tile([BLK, BLK], mybir.dt.float32)
            nc.vector.memset(warm_w, 0.0)
            for _ in range(N_WARMUP):
                nc.tensor.matmul(warm_ps, warm_w, warm_w, start=True, stop=True,
                                 skip_group_check=True)

            def load_q(b, lo_blk, hi_blk, eng):
                sl = np.s_[lo_blk:hi_blk]
                out = [eng.dma_start(out=q_hi[:, b, sl], in_=qhi_d[b, :, sl])]
                if qk_mode == "hilo":
                    out.append(eng.dma_start(out=q_lo[:, b, sl], in_=qlo_d[b, :, sl]))
                return out

            def load_k(b, lo_blk, hi_blk, eng):
                sl = np.s_[lo_blk * BLK:hi_blk * BLK]
                out = [eng.dma_start(out=k_hi[:, b, sl], in_=khi_d[b, :, sl])]
                if qk_mode == "hilo":
                    out.append(eng.dma_start(out=k_lo[:, b, sl], in_=klo_d[b, :, sl]))
                return out

            def load_v(b, lo_blk, hi_blk, eng):
                return [eng.dma_start(
                    out=v_t[:, b, lo_blk:hi_blk],
                    in_=v_d[:, b, lo_blk:hi_blk],
                )]

            # b=0 loads, ordered by first use, split over the two fast
            # DMA-issue queues (sync/SP and gpsimd). A dma_start costs
            # ~650ns of issue time, so the critical first-block set stays
            # small; everything else is chunked just enough to keep the
            # unit pipeline fed.
            load_q(0, 0, 1, nc.sync)
            load_k(0, 0, 3, nc.gpsimd)
            load_q(0, 1, 2, nc.sync)
            load_v(0, 0, 3, nc.gpsimd)
            nc.gpsimd.dma_start(out=ones_t, in_=ones_d)
            load_q(0, 2, 4, nc.sync)
            load_k(0, 3, NBLK, nc.gpsimd)
            load_v(0, 3, NBLK, nc.gpsimd)
            load_q(0, 4, NBLK, nc.sync)

            # b=1 loads: emitted now, but dependency-gated on b=0's third
            # unit so the Tile scheduler cannot hoist these 1.5MB in front
            # of b=0's critical-path transfers (it schedules dep-free DMAs
            # first, which starves the first compute units).
            b1_loads = []
            b1_loads += load_k(1, 0, NBLK, nc.gpsimd)
            b1_loads += load_v(1, 0, NBLK, nc.gpsimd)
            b1_loads += load_q(1, 0, 4, nc.scalar)
            b1_loads += load_q(1, 4, NBLK, nc.scalar)

            for b in range(B):
                # b=1 runs its lightest unit (tj0: diag only) last so the
                # final pipeline drain is as short as possible; b=0 keeps
                # ascending order to match the input DMA arrival order.
                tj_order = list(range(NBLK)) if b == 0 else list(range(1, NBLK)) + [0]
                for unit_i, tj in enumerate(tj_order):
                    entries = scheds[b][tj]
                    outp = ps_o.tile([D, HLOC, BLK], mybir.dt.float32)
                    sm = ps_m.tile([1, HLOC, BLK], mybir.dt.float32)
                    n_e = len(entries)
                    pts = []
                    for idx, (si, (klo, khi), sels, diag, tw) in enumerate(entries):
                        st = ps_s.tile([BLK, HLOC, BLK], mybir.dt.float32)
                        kh = k_hi[:, b, si * BLK:(si + 1) * BLK]
                        qh = q_hi[:, b, tj]
                        if qk_mode == "hilo":
                            kl = k_lo[:, b, si * BLK:(si + 1) * BLK]
                            ql = q_lo[:, b, tj]
                            nc.tensor.matmul(st[:, :, :tw], kh, qh[:, :, :tw],
                                             start=True, stop=False,
                                             skip_group_check=True)
                            nc.tensor.matmul(st[:, :, :tw], kh, ql[:, :, :tw],
                                             start=False, stop=False,
                                             skip_group_check=True)
                            qk_mm = nc.tensor.matmul(st[:, :, :tw], kl,
                                                     qh[:, :, :tw],
                                                     start=False, stop=True,
                                                     skip_group_check=True)
                        else:
                            qk_mm = nc.tensor.matmul(st[:, :, :tw], kh,
                                                     qh[:, :, :tw],
                                                     start=True, stop=True,
                                                     skip_group_check=True)
                        if b == 0 and tj == 2 and idx == 0:
                            for ld in b1_loads:
                                add_dep_helper(ld.ins, qk_mm.ins, sync=True,
                                               reason="b1 loads after b0 ramp")

                        # exp(x - 30): headroom against fp32 exp overflow for
                        # unlucky logit maxima; cancels in out/sums exactly.
                        pt = ptp.tile([BLK, HLOC, BLK], DT.bfloat16)
                        nc.scalar.activation(
                            out=pt[:, :, :tw], in_=st[:, :, :tw],
                            func=mybir.ActivationFunctionType.Exp,
                            bias=exp_bias,
                        )
                        if diag:
                            # keep s <= t for every head: iota = -4x + h + 4y,
                            # >= 0 iff y >= x (h in 0..3 can't flip it)
                            nc.gpsimd.affine_select(
                                out=pt, in_=pt, compare_op=mybir.AluOpType.is_ge,
                                fill=0.0, base=0,
                                pattern=[[1, HLOC], [HLOC, BLK]],
                                channel_multiplier=-HLOC,
                            )
                        for (a, e, bound, kind) in sels:
                            # zero key rows outside a t-span's segment window
                            e = min(e, tw)
                            if kind == "ge":  # keep s >= bound
                                base, cmul = -bound, 1
                            else:  # keep s < bound
                                base, cmul = bound - 1, -1
                            nc.gpsimd.affine_select(
                                out=pt[:, :, a:e], in_=pt[:, :, a:e],
                                compare_op=mybir.AluOpType.is_ge,
                                fill=0.0, base=base,
                                pattern=[[0, HLOC], [0, e - a]],
                                channel_multiplier=cmul,
                            )

                        first, last = idx == 0, idx == n_e - 1
                        nc.tensor.matmul(outp[:, :, :tw], v_t[klo:khi, b, si],
                                         pt[klo:khi, :, :tw],
                                         start=first, stop=last,
                                         skip_group_check=True)
                        pts.append((pt, klo, khi, tw))

                    # sums back-to-back after the si loop: the ones weights
                    # stay loaded across consecutive same-range matmuls
                    for j, (ptt, klo, khi, tw) in enumerate(pts):
                        nc.tensor.matmul(sm[:, :, :tw], ones_t[klo:khi],
                                         ptt[klo:khi, :, :tw],
                                         start=j == 0, stop=j == len(pts) - 1,
                                         skip_group_check=True)

                    o_sb = stage.tile([D, W], mybir.dt.float32)
                    nc.vector.tensor_copy(out=o_sb, in_=outp)
                    # per-unit sums staging + DMA (2KB) keeps the drain tail
                    # short: no end-of-batch copy chain
                    s_sb = sumstage.tile([1, W], mybir.dt.float32)
                    nc.vector.tensor_copy(out=s_sb, in_=sm)
                    off = (b * NBLK + tj) * W
                    nc.sync.dma_start(out=sums_d[:, off:off + W], in_=s_sb)
                    if b == 1 and unit_i == NBLK - 1:
                        # final unit: split the output DMA across both fast
                        # queues so the drain tail is as short as possible
                        nc.sync.dma_start(out=outT_d[b, tj][:, :W // 2],
                                          in_=o_sb[:, :W // 2])
                        nc.gpsimd.dma_start(out=outT_d[b, tj][:, W // 2:],
                                            in_=o_sb[:, W // 2:])
                    else:
                        nc.sync.dma_start(out=outT_d[b, tj], in_=o_sb)
    nc.compile()
    return nc


def kernel(query, key, value, decoder_segment_ids):
    from concourse.bass_utils import run_bass_kernel_spmd

    query = np.asarray(query, dtype=np.float32)
    key = np.asarray(key, dtype=np.float32)
    value = np.asarray(value, dtype=np.float32)
    seg = np.asarray(decoder_segment_ids, dtype=np.int32)

    scheds = [_segment_structure(seg[b]) for b in range(B)]
    sig = tuple(
        tuple(tuple(e for e in entries)
              for entries in sched)
        for sched in scheds
    ) + (QK_MODE, N_WARMUP)
    nc = _compiled_cache.get(sig)
    if nc is None:
        nc = _build_program(scheds, QK_MODE)
        _compiled_cache[sig] = nc

    ones_in = np.ones((BLK, 1), dtype=ml_dtypes.bfloat16)

    in_maps = []
    for c in range(N_CORES):
        q_c = query[:, :, c * HLOC:(c + 1) * HLOC, :]  # (B, T, HLOC, D)
        # -> (B, D, NBLK, HLOC, BLK): element [b,d,tj,h,y] = q_c[b, tj*128+y, h, d]
        qT = np.ascontiguousarray(
            q_c.transpose(0, 3, 1, 2)  # (B, D, T, HLOC)
            .reshape(B, D, NBLK, BLK, HLOC)
            .transpose(0, 1, 2, 4, 3)
        )
        kT = np.ascontiguousarray(key[:, :, c, :].transpose(0, 2, 1))  # (B, D, S)
        v_c = np.ascontiguousarray(
            value[:, :, c, :].reshape(B, NBLK, BLK, D).transpose(2, 0, 1, 3)
        ).astype(ml_dtypes.bfloat16)
        m = {"v": v_c, "ones_in": ones_in}
        if QK_MODE == "hilo":
            m["qhi"], m["qlo"] = _split_bf16(qT)
            m["khi"], m["klo"] = _split_bf16(kT)
        else:
            m["qhi"] = qT.astype(np.float16)
            m["khi"] = kT.astype(np.float16)
        in_maps.append(m)

    kwargs = {}
    if TRACE:
        kwargs = dict(trace=True, tmpdir=TRACE_DIR)
    res = run_bass_kernel_spmd(nc, in_maps, core_ids=list(range(N_CORES)), **kwargs)
    kernel.last_results = res

    out = np.empty((B, T, NQ, D), dtype=np.float32)
    for c in range(N_CORES):
        outT = res.results[c]["outT"]  # (B, NBLK, D, W) with W = (HLOC, BLK)
        sums = res.results[c]["sums"]  # (1, B*NBLK*W)
        o = outT.reshape(B, NBLK, D, HLOC, BLK)
        s = sums.reshape(B, NBLK, HLOC, BLK)
        # out[b, tj*128+y, c*4+h, d] = o[b, tj, d, h, y] / s[b, tj, h, y]
        o = o.transpose(0, 1, 4, 3, 2).reshape(B, T, HLOC, D)
        s = s.transpose(0, 1, 3, 2).reshape(B, T, HLOC)
        out[:, :, c * HLOC:(c + 1) * HLOC, :] = o / s[:, :, :, None]
    return out
